# revision 1
# baseline (speedup 1.0000x reference)
"""Trainium2 Bass kernel for nn_Decoder (MusicVAE-style hierarchical LSTM decoder).

Strategy: 8-way model parallelism over the 4096-wide LSTM gate dimension
(512 gates per core), with all 64 batch rows on every core. Weights stay
SBUF-resident. The recurrent hidden states are exchanged every step with
remote_dma_broadcast (SBUF->SBUF across the 8 NeuronCores). Matmuls run in
float32r (full-rate fp32 path). The sequential structure (conductor: 16
steps; decoder: 512 autoregressive steps) is fully unrolled.

Layout notes:
- All matmuls use out[batch=64, N] = lhsT.T @ rhs with the activations
  (transposed) stationary and weight tiles streaming, so the weight matrix
  is the moving operand (batch is only 64).
- Gate order within a core's 512-gate slice is [i f g o] x 128. The g-gate
  weights/biases are pre-doubled on the host so a single sigmoid pass over
  all 512 gate columns yields tanh(g) = 2*sigmoid(2g)-1 with one extra
  tensor_scalar op.
- note/emb projections are computed transposed (out[rows, batch]) and
  sharded 8 ways, so each core's slice is exchange-ready without an extra
  PE transpose.
"""

import os
import sys
import time

for _p in ("/opt/trn_rl_repo", "/root/.axon_site/_ro/trn_rl_repo"):
    if os.path.isdir(_p) and _p not in sys.path:
        sys.path.insert(0, _p)
        break

import numpy as np

from concourse import bass, mybir, bacc

F32 = mybir.dt.float32
F32R = mybir.dt.float32r

NC = 8           # cores
B = 64           # batch
H = 1024         # decoder hidden
HC = 1024        # conductor hidden
LATENT = 512
INPUT = 389
INPUT_PAD = 512
COND_OUT = 512
GSL = 512        # per-core gate slice (4*H/NC)
KT_H = 8         # K tiles of 128 over H
KT_L = 4         # K tiles of 128 over LATENT
SL = 64          # slot width (columns) in gathered buffers

RD = [(0, k) for k in range(NC)]


def build(nsub, nnotes, full_out=True):
    """Build the SPMD Bass program. nsub conductor steps, nsub*nnotes decoder steps."""
    CT = nsub
    T = nsub * nnotes
    TOUT = T if full_out else 1
    nc = bacc.Bacc(num_devices=NC)

    # ---------------- DRAM parameters (per-core data) ----------------
    dp = nc.declare_dram_parameter
    latT_d = dp("latT", [128, KT_L * SL], F32R, isOutput=False)
    h0T_d = dp("h0T", [nsub, 128, 2 * KT_H * SL], F32R, isOutput=False)
    c0s_d = dp("c0s", [B, nsub * 2 * 128], F32, isOutput=False)
    wx0_d = dp("wx0", [64, 8 * GSL], F32R, isOutput=False)
    wh0_d = dp("wh0", [128, 8 * GSL], F32R, isOutput=False)
    wx1_d = dp("wx1", [128, 8 * GSL], F32R, isOutput=False)
    wh1_d = dp("wh1", [128, 8 * GSL], F32R, isOutput=False)
    wdoT_d = dp("wdoT", [128, 8 * SL], F32R, isOutput=False)
    wemb_d = dp("wemb", [64, 8 * GSL], F32R, isOutput=False)
    wxc0_d = dp("wxc0", [128, KT_L * GSL], F32R, isOutput=False)
    whc0_d = dp("whc0", [128, 8 * GSL], F32R, isOutput=False)
    wxc1_d = dp("wxc1", [128, 8 * GSL], F32R, isOutput=False)
    whc1_d = dp("whc1", [128, 8 * GSL], F32R, isOutput=False)
    wcoT_d = dp("wcoT", [128, 8 * SL], F32R, isOutput=False)
    b0_d = dp("b0r", [1, GSL], F32R, isOutput=False)       # bih+bhh d0 (g x2)
    b1_d = dp("b1r", [1, GSL], F32R, isOutput=False)
    bdo_d = dp("bdoc", [1, SL], F32R, isOutput=False)      # proj bias (note-slice)
    bc0_d = dp("bc0r", [1, GSL], F32R, isOutput=False)
    bc1_d = dp("bc1r", [1, GSL], F32R, isOutput=False)
    bco_d = dp("bcoc", [1, SL], F32R, isOutput=False)      # emb bias (emb-slice)
    ones_d = dp("onesr", [1, SL], F32R, isOutput=False)
    id64_d = dp("id64", [64, 64], F32R, isOutput=False)
    idT_d = dp("idT", [128, 128], F32R, isOutput=False)
    out_d = dp("out", [TOUT, 64, 64], F32, isOutput=True)
    est_d = nc.dram_tensor("est", [nsub, B, GSL], F32R)

    import contextlib
    with contextlib.ExitStack() as ctx:
        e = ctx.enter_context
        sb = lambda name, shape, dt=F32: e(nc.sbuf_tensor(name, shape, dt))
        ps = lambda name, shape: e(nc.psum_tensor(name, shape, F32))
        sem = lambda name: e(nc.semaphore(name))

        # weights / constants in SBUF
        LAT = sb("LAT", [128, KT_L * SL], F32R)
        H0T = sb("H0T", [128, 2 * (2 * KT_H * SL)], F32R)   # ping-pong per subseq
        C0SUB = sb("C0SUB", [B, 2 * 256])
        WX0 = sb("WX0", [64, 8 * GSL], F32R)
        WH0 = sb("WH0", [128, 8 * GSL], F32R)
        WX1 = sb("WX1", [128, 8 * GSL], F32R)
        WH1 = sb("WH1", [128, 8 * GSL], F32R)
        WDOT = sb("WDOT", [128, 8 * SL], F32R)
        WEMB = sb("WEMB", [64, 8 * GSL], F32R)
        WXC0 = sb("WXC0", [128, KT_L * GSL], F32R)
        WHC0 = sb("WHC0", [128, 8 * GSL], F32R)
        WXC1 = sb("WXC1", [128, 8 * GSL], F32R)
        WHC1 = sb("WHC1", [128, 8 * GSL], F32R)
        WCOT = sb("WCOT", [128, 8 * SL], F32R)
        B0 = sb("B0", [1, GSL], F32R)
        B1 = sb("B1", [1, GSL], F32R)
        BDO = sb("BDO", [1, SL], F32R)
        BC0 = sb("BC0", [1, GSL], F32R)
        BC1 = sb("BC1", [1, GSL], F32R)
        BCO = sb("BCO", [1, SL], F32R)
        ONES = sb("ONES", [1, SL], F32R)
        ID64 = sb("ID64", [64, 64], F32R)
        IDT = sb("IDT", [128, 128], F32R)
        EBUF = sb("EBUF", [B, 2 * GSL], F32R)
        EDEC = sb("EDEC", [B, 2 * GSL], F32R)

        # gathered state buffers (ping-pong x2)
        HD0 = [sb(f"HD0_{i}", [128, NC * SL], F32R) for i in range(2)]
        HD1 = [sb(f"HD1_{i}", [128, NC * SL], F32R) for i in range(2)]
        NT = [sb(f"NT_{i}", [128, NC * SL], F32R) for i in range(2)]
        HC0 = [sb(f"HC0_{i}", [128, NC * SL], F32R) for i in range(2)]
        HC1 = [sb(f"HC1_{i}", [128, NC * SL], F32R) for i in range(2)]
        EMBT = [sb(f"EMBT_{i}", [128, NC * SL], F32R) for i in range(2)]

        # staging for outgoing tiles
        HSTG0 = [sb(f"HSTG0_{i}", [128, SL], F32R) for i in range(2)]
        HSTG1 = [sb(f"HSTG1_{i}", [128, SL], F32R) for i in range(2)]
        SNT = [sb(f"SNT_{i}", [128, SL], F32R) for i in range(2)]
        SEM_ = [sb(f"SEM_{i}", [128, SL], F32R) for i in range(2)]

        # activation work tiles (layer 0/1; conductor reuses the same)
        S0 = sb("S0", [B, GSL])
        S1 = sb("S1", [B, GSL])
        CC0 = sb("CC0", [B, 256])   # [g' | c] layer0 (decoder)
        CC1 = sb("CC1", [B, 256])
        CCC0 = sb("CCC0", [B, 256])  # conductor cell states
        CCC1 = sb("CCC1", [B, 256])
        TMP0 = sb("TMP0", [B, 256])
        TMP1 = sb("TMP1", [B, 256])
        TT0 = sb("TT0", [B, 128])
        TT1 = sb("TT1", [B, 128])
        HT0 = sb("HT0", [B, 128], F32R)   # h tiles (pre-transpose)
        HT1 = sb("HT1", [B, 128], F32R)

        # psum
        psd0 = [ps(f"psd0_{i}", [64, GSL]) for i in range(2)]
        psd1 = [ps(f"psd1_{i}", [64, GSL]) for i in range(2)]
        pspr = ps("pspr", [64, 64])
        pstr0 = ps("pstr0", [128, 64])
        pstr1 = ps("pstr1", [128, 64])
        psem = ps("psem", [64, GSL])

        # semaphores
        dw = sem("dw"); dh = sem("dh"); gi = sem("gi")
        do = [sem("doa"), sem("dob")]
        de = [sem("dea"), sem("deb")]; ep = [sem("epa"), sem("epb")]
        pe_s = sem("pe_s"); act_s = sem("act_s"); dve_s = sem("dve_s")
        r_h0 = sem("r_h0"); r_h1 = sem("r_h1"); r_nt = sem("r_nt"); r_em = sem("r_em")
        l_h0 = [sem("l_h0a"), sem("l_h0b")]; l_h1 = [sem("l_h1a"), sem("l_h1b")]
        l_nt = [sem("l_nta"), sem("l_ntb")]; l_em = [sem("l_ema"), sem("l_emb")]
        prep = sem("prep")

        N_MEMSET = 12  # NT/SNT/SEM_/EMBT x2, HC0[1], HC1[1], CCC0/1
        N_WLOAD = 21   # dma_start count on sync at init (x16 each)

        # ---- sem threshold helpers (single source of truth) ----
        def pe_c(ct, k):      # conductor: d0c=1, tr0=2, d1c=3, tr1=4, em=5, E=6
            return 6 * ct + k

        def pe_d(t, k):       # decoder: d0=1, tr0=2, d1=3, tr1=4, pr=5
            return 6 * CT + 5 * t + k

        def act_c(ct, k):     # sig0=1, tanh0=2, sig1=3, tanh1=4, tanhP=5
            return 5 * ct + k

        def act_d(t, k):
            return 5 * CT + 5 * t + k

        def dve_c(ct, k):     # g'0=1 ti=2 tf=3 cn=4 h0=5 cp0=6 g'1=7 ti=8 tf=9 cn=10 h1=11 cp1=12 E=13
            return 13 * ct + k

        def dve_d(t, k):      # same minus E: 12/step
            return 13 * CT + 12 * t + k

        def snd_c(ct):        # sends of parity-(ct%2) staging strictly before conductor step ct
            p = ct % 2
            return (ct - p) // 2

        def snd_d(t):         # conductor sends of this parity + decoder sends before t
            p = t % 2
            return (CT - p + 1) // 2 + (t - p) // 2

        with nc.Block() as block:

            # ================= SYNC: DMAs =================
            @block.sync
            def _(sy):
                loads = [
                    (LAT, latT_d),
                    (WX0, wx0_d), (WH0, wh0_d), (WX1, wx1_d), (WH1, wh1_d),
                    (WDOT, wdoT_d), (WEMB, wemb_d),
                    (WXC0, wxc0_d), (WHC0, whc0_d), (WXC1, wxc1_d), (WHC1, whc1_d),
                    (WCOT, wcoT_d),
                    (B0, b0_d), (B1, b1_d), (BDO, bdo_d),
                    (BC0, bc0_d), (BC1, bc1_d), (BCO, bco_d),
                    (ONES, ones_d), (ID64, id64_d), (IDT, idT_d),
                ]
                for dst, src in loads:
                    sy.dma_start(out=dst[:, :], in_=src[:, :]).then_inc(dw, 16)
                # first subsequence h/c init
                sy.dma_start(out=H0T[:, 0:2 * KT_H * SL], in_=h0T_d[0, :, :]).then_inc(dh, 16)
                sy.dma_start(out=C0SUB[:, 0:256], in_=c0s_d[:, 0:256]).then_inc(dh, 16)

                # conductor: store E_s to DRAM scratch
                for ct in range(CT):
                    sy.wait_ge(dve_s, dve_c(ct, 13))
                    sy.dma_start(out=est_d[ct], in_=EBUF[:, GSL * (ct % 2):GSL * (ct % 2 + 1)]).then_inc(de[ct % 2], 16)

                def n_stores(par):
                    return len([c for c in range(CT) if c % 2 == par])

                # first E prefetch (s=0)
                sy.wait_ge(de[0], 16 * n_stores(0))
                sy.dma_start(out=EDEC[:, 0:GSL], in_=est_d[0]).then_inc(ep[0], 16)

                # decoder phase: per-subsequence prefetch + output DMA
                for t in range(T):
                    s, n = divmod(t, nnotes)
                    if n == 2 and s + 1 < nsub:
                        sy.wait_ge(pe_s, pe_d(t - 1, 5))
                        sp = (s + 1) % 2
                        sy.dma_start(
                            out=H0T[:, sp * (2 * KT_H * SL):(sp + 1) * (2 * KT_H * SL)],
                            in_=h0T_d[s + 1, :, :],
                        ).then_inc(dh, 16)
                        sy.dma_start(out=C0SUB[:, sp * 256:sp * 256 + 256],
                                     in_=c0s_d[:, (s + 1) * 256:(s + 2) * 256]).then_inc(dh, 16)
                        sy.wait_ge(de[sp], 16 * n_stores(sp))
                        sy.dma_start(out=EDEC[:, sp * GSL:(sp + 1) * GSL], in_=est_d[s + 1]).then_inc(ep[sp], 16)
                    p = t % 2
                    sy.wait_ge(act_s, act_d(t, 5))
                    sy.dma_start(out=out_d[t if full_out else 0],
                                 in_=SNT[p][0:64, :].bitcast(F32)).then_inc(do[p], 16)

            # ================= GPSIMD: memsets + exchanges =================
            @block.gpsimd
            def _(g):
                U32 = mybir.dt.uint32
                for tile in (NT[0], NT[1], EMBT[0], EMBT[1]):
                    g.memset(tile[:, :].bitcast(U32), 0).then_inc(gi, 1)
                for tile in (SNT[0], SNT[1], SEM_[0], SEM_[1], HC0[1], HC1[1]):
                    g.memset(tile[:, :].bitcast(U32), 0).then_inc(gi, 1)
                g.memset(CCC0[:, 128:256].bitcast(U32), 0).then_inc(gi, 1)
                g.memset(CCC1[:, 128:256].bitcast(U32), 0).then_inc(gi, 1)
                g.wait_ge(gi, N_MEMSET)
                pid = g.partition_id()
                off = g.scalar_reg_alu(mybir.AluOpType.mult, pid, SL)
                np_ = [0]

                def step_bcasts(specs):
                    # prepare all descriptors first (desc-gen off critical path),
                    # then fire triggers in FIFO order as data becomes ready
                    for stg, gath, rsem, lsem, _, _ in specs:
                        g.remote_dma_broadcast(
                            out_ap=gath[:, bass.ds(off, SL)], in_ap=stg[:, :],
                            remote_sem=rsem, local_sem=lsem, rdests=RD,
                        ).then_inc(prep, 1)
                        np_[0] += 1
                    g.wait_ge(prep, np_[0])
                    for _, _, _, _, wait_sem, wait_val in specs:
                        g.wait_ge(wait_sem, wait_val)
                        g.trigger_dma(count=1)

                for ct in range(CT):
                    p = ct % 2
                    step_bcasts([
                        (HSTG0[p], HC0[p], r_h0, l_h0[p], dve_s, dve_c(ct, 6)),
                        (HSTG1[p], HC1[p], r_h1, l_h1[p], dve_s, dve_c(ct, 12)),
                        (SEM_[p], EMBT[p], r_em, l_em[p], act_s, act_c(ct, 5)),
                    ])
                for t in range(T):
                    p = t % 2
                    step_bcasts([
                        (HSTG0[p], HD0[p], r_h0, l_h0[p], dve_s, dve_d(t, 6)),
                        (HSTG1[p], HD1[p], r_h1, l_h1[p], dve_s, dve_d(t, 12)),
                        (SNT[p], NT[p], r_nt, l_nt[p], act_s, act_d(t, 5)),
                    ])

            # ================= TENSOR: matmuls + transposes =================
            @block.tensor
            def _(t_):
                def mm(out, lhsT, rhs, first, last, inc=None):
                    m = t_.matmul(out, lhsT, rhs, start=first, stop=last)
                    if inc is not None:
                        m.then_inc(inc, 1)
                    return m

                t_.wait_ge(dw, 16 * N_WLOAD)
                t_.wait_ge(gi, N_MEMSET)

                # ---------- conductor ----------
                for ct in range(CT):
                    p, p1 = ct % 2, (ct - 1) % 2
                    # layer c0: psd0[p] = bc0 + latent@WXC0 + hc0(ct-1)@WHC0
                    if ct >= 2:
                        t_.wait_ge(act_s, act_c(ct - 2, 1))
                    mm(psd0[p][:, :], ONES[:, :], BC0[:, :], True, False)
                    for k in range(KT_L):
                        mm(psd0[p][:, :], LAT[:, SL * k:SL * (k + 1)],
                           WXC0[:, GSL * k:GSL * (k + 1)], False, False)
                    if ct >= 1:
                        t_.wait_ge(r_h0, 16 * ct)
                    for k in range(KT_H):
                        mm(psd0[p][:, :], HC0[p1][:, SL * k:SL * (k + 1)],
                           WHC0[:, GSL * k:GSL * (k + 1)], False, k == KT_H - 1,
                           inc=pe_s if k == KT_H - 1 else None)
                    # transpose hc0 tile
                    t_.wait_ge(dve_s, dve_c(ct, 5))
                    t_.transpose(pstr0[:, :].bitcast(F32R), HT0[:, :], IDT[0:64, 0:64]).then_inc(pe_s, 1)
                    # layer c1
                    if ct >= 2:
                        t_.wait_ge(act_s, act_c(ct - 2, 3))
                    mm(psd1[p][:, :], ONES[:, :], BC1[:, :], True, False)
                    if ct >= 1:
                        t_.wait_ge(r_h1, 16 * ct)
                    for k in range(KT_H):
                        mm(psd1[p][:, :], HC1[p1][:, SL * k:SL * (k + 1)],
                           WHC1[:, GSL * k:GSL * (k + 1)], False, False)
                    t_.wait_ge(r_h0, 16 * (ct + 1))
                    for k in range(KT_H):
                        mm(psd1[p][:, :], HC0[p][:, SL * k:SL * (k + 1)],
                           WXC1[:, GSL * k:GSL * (k + 1)], False, k == KT_H - 1,
                           inc=pe_s if k == KT_H - 1 else None)
                    t_.wait_ge(dve_s, dve_c(ct, 11))
                    t_.transpose(pstr1[:, :].bitcast(F32R), HT1[:, :], IDT[0:64, 0:64]).then_inc(pe_s, 1)
                    # emb projection (transposed, sharded): pspr = bco + Wco_slice @ hc1.T
                    if ct >= 1:
                        t_.wait_ge(act_s, act_c(ct - 1, 5))
                    mm(pspr[:, :], BCO[:, :], ONES[:, :], True, False)
                    t_.wait_ge(r_h1, 16 * (ct + 1))
                    for k in range(KT_H):
                        mm(pspr[:, :], WCOT[:, SL * k:SL * (k + 1)],
                           HC1[p][:, SL * k:SL * (k + 1)], False, k == KT_H - 1,
                           inc=pe_s if k == KT_H - 1 else None)
                    # E_s = b0 + emb@Wemb-part  (uses gathered EMBT)
                    if ct >= 2:
                        t_.wait_ge(dve_s, dve_c(ct - 2, 13))
                    mm(psem[:, :], ONES[:, :], B0[:, :], True, False)
                    t_.wait_ge(r_em, 16 * (ct + 1))
                    for j in range(8):
                        mm(psem[:, :], EMBT[p][0:64, SL * j:SL * (j + 1)],
                           WEMB[0:64, GSL * j:GSL * (j + 1)], False, j == 7,
                           inc=pe_s if j == 7 else None)

                # ---------- decoder ----------
                for t in range(T):
                    p, p1 = t % 2, (t - 1) % 2
                    s, n = divmod(t, nnotes)
                    sb_ = s % 2
                    h0base = sb_ * (2 * KT_H * SL)
                    # ---- layer d0 ----
                    if t >= 2:
                        t_.wait_ge(act_s, act_d(t - 2, 1))
                    else:
                        t_.wait_ge(act_s, act_c(CT - 2 + t, 1))
                    if n == 0:
                        t_.wait_ge(ep[s % 2], 16 * ((s - s % 2) // 2 + 1))
                    mm(psd0[p][:, :], ID64[:, :], EDEC[:, GSL * (s % 2):GSL * (s % 2 + 1)], True, False)
                    if n == 0:
                        t_.wait_ge(dh, 32 * (s + 1))
                        stat = lambda k: H0T[:, h0base + SL * k:h0base + SL * (k + 1)]
                    else:
                        stat = lambda k: HD0[p1][:, SL * k:SL * (k + 1)]
                    for k in range(KT_H):
                        mm(psd0[p][:, :], stat(k), WH0[:, GSL * k:GSL * (k + 1)], False, False)
                    if t >= 1:
                        t_.wait_ge(r_nt, 16 * t)
                    for j in range(8):
                        mm(psd0[p][:, :], NT[p1][0:64, SL * j:SL * (j + 1)],
                           WX0[0:64, GSL * j:GSL * (j + 1)], False, j == 7,
                           inc=pe_s if j == 7 else None)
                    t_.wait_ge(dve_s, dve_d(t, 5))
                    t_.transpose(pstr0[:, :].bitcast(F32R), HT0[:, :], IDT[0:64, 0:64]).then_inc(pe_s, 1)
                    # ---- layer d1 ----
                    if t >= 2:
                        t_.wait_ge(act_s, act_d(t - 2, 3))
                    else:
                        t_.wait_ge(act_s, act_c(CT - 2 + t, 3))
                    mm(psd1[p][:, :], ONES[:, :], B1[:, :], True, False)
                    if n == 0:
                        stat1 = lambda k: H0T[:, h0base + (KT_H + k) * SL:h0base + (KT_H + k + 1) * SL]
                    else:
                        stat1 = lambda k: HD1[p1][:, SL * k:SL * (k + 1)]
                    for k in range(KT_H):
                        mm(psd1[p][:, :], stat1(k), WH1[:, GSL * k:GSL * (k + 1)], False, False)
                    t_.wait_ge(r_h0, 16 * (CT + t + 1))
                    for k in range(KT_H):
                        mm(psd1[p][:, :], HD0[p][:, SL * k:SL * (k + 1)],
                           WX1[:, GSL * k:GSL * (k + 1)], False, k == KT_H - 1,
                           inc=pe_s if k == KT_H - 1 else None)
                    t_.wait_ge(dve_s, dve_d(t, 11))
                    t_.transpose(pstr1[:, :].bitcast(F32R), HT1[:, :], IDT[0:64, 0:64]).then_inc(pe_s, 1)
                    # ---- note projection (transposed, sharded) ----
                    if t >= 1:
                        t_.wait_ge(act_s, act_d(t - 1, 5))
                    else:
                        t_.wait_ge(act_s, act_c(CT - 1, 5))
                    mm(pspr[:, :], BDO[:, :], ONES[:, :], True, False)
                    t_.wait_ge(r_h1, 16 * (CT + t + 1))
                    for k in range(KT_H):
                        mm(pspr[:, :], WDOT[:, SL * k:SL * (k + 1)],
                           HD1[p][:, SL * k:SL * (k + 1)], False, k == KT_H - 1,
                           inc=pe_s if k == KT_H - 1 else None)

            # ================= SCALAR (ACT) =================
            @block.scalar
            def _(a):
                SIG = mybir.ActivationFunctionType.Sigmoid
                TANH = mybir.ActivationFunctionType.Tanh

                def layer_acts(pe_done, dve_cn, dve_hprev, S, CC, TTt, psrc):
                    a.wait_ge(pe_s, pe_done)
                    if dve_hprev is not None:
                        a.wait_ge(dve_s, dve_hprev)
                    a.activation(S[:, :], psrc[:, :], SIG).then_inc(act_s, 1)
                    a.wait_ge(dve_s, dve_cn)
                    a.activation(TTt[:, :], CC[:, 128:256], TANH).then_inc(act_s, 1)

                # conductor
                for ct in range(CT):
                    p = ct % 2
                    layer_acts(pe_c(ct, 1), dve_c(ct, 4),
                               dve_c(ct - 1, 5) if ct >= 1 else None,
                               S0, CCC0, TT0, psd0[p])
                    layer_acts(pe_c(ct, 3), dve_c(ct, 10),
                               dve_c(ct - 1, 11) if ct >= 1 else None,
                               S1, CCC1, TT1, psd1[p])
                    a.wait_ge(pe_s, pe_c(ct, 5))
                    if snd_c(ct) > 0:
                        a.wait_ge(l_em[ct % 2], 16 * snd_c(ct))
                    a.activation(SEM_[p][0:64, :], pspr[:, :], TANH).then_inc(act_s, 1)
                # decoder
                for t in range(T):
                    p = t % 2
                    layer_acts(pe_d(t, 1), dve_d(t, 4),
                               dve_d(t - 1, 5) if t >= 1 else dve_c(CT - 1, 5),
                               S0, CC0, TT0, psd0[p])
                    layer_acts(pe_d(t, 3), dve_d(t, 10),
                               dve_d(t - 1, 11) if t >= 1 else dve_c(CT - 1, 11),
                               S1, CC1, TT1, psd1[p])
                    a.wait_ge(pe_s, pe_d(t, 5))
                    if t >= 2:
                        a.wait_ge(l_nt[t % 2], 16 * ((t - t % 2) // 2))
                        a.wait_ge(do[t % 2], 16 * ((t - t % 2) // 2))
                    a.activation(SNT[p][0:64, :], pspr[:, :], TANH).then_inc(act_s, 1)

            # ================= VECTOR (DVE) =================
            @block.vector
            def _(v):
                MUL = mybir.AluOpType.mult
                ADD = mybir.AluOpType.add
                SUB = mybir.AluOpType.subtract

                def layer_chain(base, sig_done, tanh_done, tr_done, l_sem, l_val,
                                S, CC, TMP, TTt, HTt, pstr, HSTGt, c_src):
                    # g' = 2*sig(2g) - 1
                    v.wait_ge(act_s, sig_done)
                    v.tensor_scalar(CC[:, 0:128], S[:, 256:384], 2.0, 1.0, MUL, SUB).then_inc(dve_s, 1)
                    # tmp_i = S_i * g' ; tmp_f = S_f * c
                    v.wait_ge(dve_s, base + 1)
                    v.tensor_tensor(TMP[:, 0:128], S[:, 0:128], CC[:, 0:128], MUL).then_inc(dve_s, 1)
                    v.tensor_tensor(TMP[:, 128:256], S[:, 128:256], c_src, MUL).then_inc(dve_s, 1)
                    # c_new
                    v.wait_ge(dve_s, base + 3)
                    v.tensor_tensor(CC[:, 128:256], TMP[:, 0:128], TMP[:, 128:256], ADD).then_inc(dve_s, 1)
                    # h = S_o * tanh(c)
                    v.wait_ge(act_s, tanh_done)
                    v.tensor_tensor(HTt[:, :], S[:, 384:512], TTt[:, :], MUL).then_inc(dve_s, 1)
                    # copy transpose psum -> staging
                    v.wait_ge(pe_s, tr_done)
                    if l_val > 0:
                        v.wait_ge(l_sem, l_val)
                    v.tensor_copy(HSTGt[:, :], pstr[:, :].bitcast(F32R)).then_inc(dve_s, 1)

                v.wait_ge(gi, N_MEMSET)
                for ct in range(CT):
                    p = ct % 2
                    layer_chain(dve_c(ct, 0), act_c(ct, 1), act_c(ct, 2), pe_c(ct, 2),
                                l_h0[p], 16 * snd_c(ct), S0, CCC0, TMP0, TT0, HT0,
                                pstr0, HSTG0[p], CCC0[:, 128:256])
                    layer_chain(dve_c(ct, 6), act_c(ct, 3), act_c(ct, 4), pe_c(ct, 4),
                                l_h1[p], 16 * snd_c(ct), S1, CCC1, TMP1, TT1, HT1,
                                pstr1, HSTG1[p], CCC1[:, 128:256])
                    # copy E psum -> EBUF (DRAM-bounced by sync)
                    v.wait_ge(pe_s, pe_c(ct, 6))
                    if (ct - ct % 2) // 2 > 0:
                        v.wait_ge(de[ct % 2], 16 * ((ct - ct % 2) // 2))
                    v.tensor_copy(EBUF[:, GSL * (ct % 2):GSL * (ct % 2 + 1)], psem[:, :].bitcast(F32R)).then_inc(dve_s, 1)
                # decoder
                for t in range(T):
                    p = t % 2
                    s, n = divmod(t, nnotes)
                    sp_ = s % 2
                    c0src = C0SUB[:, sp_ * 256:sp_ * 256 + 128] if n == 0 else CC0[:, 128:256]
                    c1src = C0SUB[:, sp_ * 256 + 128:sp_ * 256 + 256] if n == 0 else CC1[:, 128:256]
                    if n == 0:
                        v.wait_ge(dh, 32 * (s + 1))
                    layer_chain(dve_d(t, 0), act_d(t, 1), act_d(t, 2), pe_d(t, 2),
                                l_h0[p], 16 * snd_d(t), S0, CC0, TMP0, TT0, HT0,
                                pstr0, HSTG0[p], c0src)
                    layer_chain(dve_d(t, 6), act_d(t, 3), act_d(t, 4), pe_d(t, 4),
                                l_h1[p], 16 * snd_d(t), S1, CC1, TMP1, TT1, HT1,
                                pstr1, HSTG1[p], c1src)

    nc.compile()
    return nc


# ======================= host-side preparation =======================

def _gate_slice_ixs(core):
    """Column indices (into the 4H gate dim, PyTorch i,f,g,o order) for one
    core's 512-gate slice, ordered [i(128) f(128) g(128) o(128)]."""
    ix = []
    for gg in range(4):
        base = gg * H + core * 128
        ix.extend(range(base, base + 128))
    return np.array(ix)


def prep_inputs(inputs, nsub=16, nnotes=32):
    f = lambda x: np.asarray(x, dtype=np.float32)
    latent = f(inputs["latent"])
    h0_dec = f(inputs["h0_dec"])[:nsub]
    c0_dec = f(inputs["c0_dec"])[:nsub]

    def pack_k(wT, kt):
        # wT: [K, N] -> [128, kt*N] tiles along K
        K, N = wT.shape
        assert K == kt * 128
        out = np.empty((128, kt * N), np.float32)
        for k in range(kt):
            out[:, N * k:N * (k + 1)] = wT[128 * k:128 * (k + 1), :]
        return out

    def pack_k64(wT, kt):
        K, N = wT.shape
        assert K == kt * 64
        out = np.empty((64, kt * N), np.float32)
        for k in range(kt):
            out[:, N * k:N * (k + 1)] = wT[64 * k:64 * (k + 1), :]
        return out

    # h0T packed: [s, p, (l k b)]
    h0T = np.einsum("slbk->slkb", h0_dec)  # [s, l, 1024, 64]
    h0T_packed = np.empty((nsub, 128, 2 * KT_H * SL), np.float32)
    for s in range(nsub):
        for l in range(2):
            for k in range(KT_H):
                h0T_packed[s, :, (l * KT_H + k) * SL:(l * KT_H + k + 1) * SL] = \
                    h0T[s, l, 128 * k:128 * (k + 1), :]

    latT = np.ascontiguousarray(latent.T)  # [512, 64]
    latT_packed = pack_k(latT, KT_L)

    ident64 = np.eye(64, dtype=np.float32)
    identT = np.eye(128, dtype=np.float32)
    ones_row = np.ones((1, SL), np.float32)

    Wih_d0, Whh_d0 = f(inputs["Wih_d0"]), f(inputs["Whh_d0"])
    Wih_d1, Whh_d1 = f(inputs["Wih_d1"]), f(inputs["Whh_d1"])
    Wdo, bdo = f(inputs["Wdo"]), f(inputs["bdo"])
    Wih_c0, Whh_c0 = f(inputs["Wih_c0"]), f(inputs["Whh_c0"])
    Wih_c1, Whh_c1 = f(inputs["Wih_c1"]), f(inputs["Whh_c1"])
    Wco, bco = f(inputs["Wco"]), f(inputs["bco"])
    b0_full = f(inputs["bih_d0"]) + f(inputs["bhh_d0"])
    b1_full = f(inputs["bih_d1"]) + f(inputs["bhh_d1"])
    bc0_full = f(inputs["bih_c0"]) + f(inputs["bhh_c0"])
    bc1_full = f(inputs["bih_c1"]) + f(inputs["bhh_c1"])

    Wdo_pad = np.zeros((INPUT_PAD, H), np.float32)
    Wdo_pad[:INPUT] = Wdo
    bdo_pad = np.zeros(INPUT_PAD, np.float32)
    bdo_pad[:INPUT] = bdo

    in_maps = []
    for core in range(NC):
        ix = _gate_slice_ixs(core)
        gmask = np.ones(GSL, np.float32)
        gmask[256:384] = 2.0  # double g-gate pre-activations

        def slc(w, xdim=None):
            # w: [4H, K] -> [K, 512] slice with g-doubling
            wT = w[ix, :].T.astype(np.float32) * gmask[None, :]
            return np.ascontiguousarray(wT)

        wx0_full = np.zeros((INPUT_PAD, GSL), np.float32)
        wx0_full[:INPUT] = slc(Wih_d0[:, :INPUT])
        wemb_full = slc(Wih_d0[:, INPUT:INPUT + COND_OUT])  # [512, 512]

        m = {
            "latT": latT_packed,
            "h0T": h0T_packed,
            "c0s": np.ascontiguousarray(
                c0_dec[:, :, :, core * 128:(core + 1) * 128].transpose(2, 0, 1, 3).reshape(B, -1)),
            "wx0": pack_k64(wx0_full, 8),
            "wh0": pack_k(slc(Whh_d0), KT_H),
            "wx1": pack_k(slc(Wih_d1), KT_H),
            "wh1": pack_k(slc(Whh_d1), KT_H),
            "wdoT": pack_k(np.ascontiguousarray(Wdo_pad.T[:, core * SL:(core + 1) * SL]), KT_H),
            "wemb": pack_k64(wemb_full, 8),
            "wxc0": pack_k(slc(Wih_c0), KT_L),
            "whc0": pack_k(slc(Whh_c0), KT_H),
            "wxc1": pack_k(slc(Wih_c1), KT_H),
            "whc1": pack_k(slc(Whh_c1), KT_H),
            "wcoT": pack_k(np.ascontiguousarray(Wco.T[:, core * SL:(core + 1) * SL]), KT_H),
            "b0r": (b0_full[ix] * gmask)[None, :],
            "b1r": (b1_full[ix] * gmask)[None, :],
            "bdoc": bdo_pad[core * SL:(core + 1) * SL][None, :],
            "bc0r": (bc0_full[ix] * gmask)[None, :],
            "bc1r": (bc1_full[ix] * gmask)[None, :],
            "bcoc": bco[core * SL:(core + 1) * SL][None, :],
            "onesr": ones_row,
            "id64": ident64,
            "idT": identT,
        }
        in_maps.append({k: np.ascontiguousarray(v, dtype=np.float32) for k, v in m.items()})
    return in_maps


def assemble_output(results, nsub=16, nnotes=32):
    T = nsub * nnotes
    # each core: out [T, 64(note rows), 64(batch)] -> concat note rows
    full = np.concatenate([results[c]["out"] for c in range(NC)], axis=1)  # [T, 512, 64]
    return np.ascontiguousarray(full[:, :INPUT, :].transpose(2, 0, 1))  # [B, T, INPUT]


_CACHED = {}


def kernel(**inputs) -> np.ndarray:
    from concourse.bass_utils import run_bass_kernel_spmd
    nsub, nnotes = 16, 32
    key = (nsub, nnotes)
    if key not in _CACHED:
        _CACHED[key] = build(nsub, nnotes)
    nc = _CACHED[key]
    in_maps = prep_inputs(inputs, nsub, nnotes)
    res = run_bass_kernel_spmd(nc, in_maps, core_ids=list(range(NC)))
    return assemble_output(res.results, nsub, nnotes)



# revision 2
# speedup vs baseline: 683.6791x; 683.6791x over previous
"""Trainium2 Bass kernel for nn_Decoder (MusicVAE-style hierarchical LSTM decoder), v2.

8-way model parallelism over the 4096-wide LSTM gate dimension (512 gates
per core). All matmul operands are bf16 (psum/cell-state stay f32). Per
step the recurrent h-slices are exchanged with remote_dma_broadcast.

v2 structural changes vs the f32r baseline:
- bf16 weights/activations halve exchange bytes and speed small-N matmuls.
- Elementwise LSTM cell math runs TRANSPOSED at full 128-lane width: the
  sigmoid output S [64,512] is transposed per-gate on the PE into psum
  ([128,64] tiles g,i,f,o), the DVE chain produces h.T directly into the
  bf16 broadcast staging buffer (no separate h transpose + copy).
- note/emb projections are batch-split across core pairs: core c computes
  note rows [128*(c//2), +128) for batch half (c%2). This makes the
  gathered note.T/emb.T buffers K=128-contractable (4 matmuls instead of
  8 K=64 ones on the critical path) and the frames [128,32].
- E_s (emb-dependent decoder gate bias) and the per-subsequence decoder
  h/c inits live SBUF-resident for the whole kernel (no DRAM bounce).
- PE issue order is software-pipelined across steps: the next step's
  x-independent gate matmuls issue before the current step's projection.
"""

import os
import sys

for _p in ("/opt/trn_rl_repo", "/root/.axon_site/_ro/trn_rl_repo"):
    if os.path.isdir(_p) and _p not in sys.path:
        sys.path.insert(0, _p)
        break

import numpy as np

from concourse import bass, mybir, bacc

F32 = mybir.dt.float32
BF16 = mybir.dt.bfloat16

NC = 8
B = 64
H = 1024
GSL = 512        # per-core gate slice
SL = 64
KT_H = 8
KT_L = 4
INPUT = 389
INPUT_PAD = 512

RD = [(0, k) for k in range(NC)]


def build(nsub, nnotes, full_out=True):
    CT = nsub
    T = nsub * nnotes
    TOUT = T if full_out else 1
    nc = bacc.Bacc(num_devices=NC)

    dp = nc.declare_dram_parameter
    latT_d = dp("latT", [128, KT_L * SL], BF16, isOutput=False)
    h0TA_d = dp("h0TA", [128, CT * 2 * KT_H * SL], BF16, isOutput=False)
    c0T_d = dp("c0T", [128, CT * 2 * SL], F32, isOutput=False)
    wx0_d = dp("wx0", [128, 4 * GSL], BF16, isOutput=False)
    wh0_d = dp("wh0", [128, 8 * GSL], BF16, isOutput=False)
    wx1_d = dp("wx1", [128, 8 * GSL], BF16, isOutput=False)
    wh1_d = dp("wh1", [128, 8 * GSL], BF16, isOutput=False)
    wdoT_d = dp("wdoT", [128, 8 * 128], BF16, isOutput=False)
    wemb_d = dp("wemb", [128, 4 * GSL], BF16, isOutput=False)
    wxc0_d = dp("wxc0", [128, KT_L * GSL], BF16, isOutput=False)
    whc0_d = dp("whc0", [128, 8 * GSL], BF16, isOutput=False)
    wxc1_d = dp("wxc1", [128, 8 * GSL], BF16, isOutput=False)
    whc1_d = dp("whc1", [128, 8 * GSL], BF16, isOutput=False)
    wcoT_d = dp("wcoT", [128, 8 * 128], BF16, isOutput=False)
    b0_d = dp("b0r", [1, GSL], BF16, isOutput=False)
    b1_d = dp("b1r", [1, GSL], BF16, isOutput=False)
    bc0_d = dp("bc0r", [1, GSL], BF16, isOutput=False)
    bc1_d = dp("bc1r", [1, GSL], BF16, isOutput=False)
    bdo_d = dp("bdoc", [1, 128], BF16, isOutput=False)
    bco_d = dp("bcoc", [1, 128], BF16, isOutput=False)
    ones_d = dp("onesr", [1, SL], BF16, isOutput=False)
    id64_d = dp("id64", [64, 64], BF16, isOutput=False)
    out_d = dp("out", [TOUT, 128, 32], BF16, isOutput=True)

    import contextlib
    with contextlib.ExitStack() as ctx:
        e = ctx.enter_context
        sb = lambda name, shape, dt=F32: e(nc.sbuf_tensor(name, shape, dt))
        ps = lambda name, shape: e(nc.psum_tensor(name, shape, F32))
        sem = lambda name: e(nc.semaphore(name))

        # weights / constants (SBUF, bf16)
        LAT = sb("LAT", [128, KT_L * SL], BF16)
        H0TA = sb("H0TA", [128, CT * 2 * KT_H * SL], BF16)
        C0T = sb("C0T", [128, CT * 2 * SL])
        WX0 = sb("WX0", [128, 4 * GSL], BF16)
        WH0 = sb("WH0", [128, 8 * GSL], BF16)
        WX1 = sb("WX1", [128, 8 * GSL], BF16)
        WH1 = sb("WH1", [128, 8 * GSL], BF16)
        WDOT = sb("WDOT", [128, 8 * 128], BF16)
        WEMB = sb("WEMB", [128, 4 * GSL], BF16)
        WXC0 = sb("WXC0", [128, KT_L * GSL], BF16)
        WHC0 = sb("WHC0", [128, 8 * GSL], BF16)
        WXC1 = sb("WXC1", [128, 8 * GSL], BF16)
        WHC1 = sb("WHC1", [128, 8 * GSL], BF16)
        WCOT = sb("WCOT", [128, 8 * 128], BF16)
        B0 = sb("B0", [1, GSL], BF16)
        B1 = sb("B1", [1, GSL], BF16)
        BC0 = sb("BC0", [1, GSL], BF16)
        BC1 = sb("BC1", [1, GSL], BF16)
        BDO = sb("BDO", [1, 128], BF16)
        BCO = sb("BCO", [1, 128], BF16)
        ONES = sb("ONES", [1, SL], BF16)
        ID64 = sb("ID64", [64, 64], BF16)
        EST = sb("EST", [B, CT * GSL], BF16)   # per-subseq emb gate-bias

        # gathered exchange buffers (ping-pong)
        HD0 = [sb(f"HD0_{i}", [128, NC * SL], BF16) for i in range(2)]
        HD1 = [sb(f"HD1_{i}", [128, NC * SL], BF16) for i in range(2)]
        NT = [sb(f"NT_{i}", [128, NC * 32], BF16) for i in range(2)]
        HC0 = [sb(f"HC0_{i}", [128, NC * SL], BF16) for i in range(2)]
        HC1 = [sb(f"HC1_{i}", [128, NC * SL], BF16) for i in range(2)]
        EMBT = [sb(f"EMBT_{i}", [128, NC * 32], BF16) for i in range(2)]

        # outgoing staging
        HSTG0 = [sb(f"HSTG0_{i}", [128, SL], BF16) for i in range(2)]
        HSTG1 = [sb(f"HSTG1_{i}", [128, SL], BF16) for i in range(2)]
        SNT = [sb(f"SNT_{i}", [128, 32], BF16) for i in range(2)]
        SEM_ = [sb(f"SEM_{i}", [128, 32], BF16) for i in range(2)]

        # activations / cell state
        S0 = sb("S0", [B, GSL], BF16)
        S1 = sb("S1", [B, GSL], BF16)
        CT0 = sb("CT0", [128, SL])     # decoder cell state, transposed
        CT1 = sb("CT1", [128, SL])
        CTC0 = sb("CTC0", [128, SL])   # conductor cell state
        CTC1 = sb("CTC1", [128, SL])
        UR = sb("UR", [B, 128], BF16)  # i*tanh(g), untransposed
        FCT = sb("FCT", [128, SL])     # f*c .T
        TT0 = sb("TT0", [128, SL], BF16)  # tanh(c).T
        TT1 = sb("TT1", [128, SL], BF16)

        # psum (gate psums split per column half so activations can read
        # one half while the other half's matmuls still accumulate)
        psd0a = ps("psd0a", [64, 256])
        psd0b = ps("psd0b", [64, 256])
        psd1a = ps("psd1a", [64, 256])
        psd1b = ps("psd1b", [64, 256])
        PSTR = ps("PSTR", [128, 128])          # bf16 view: [128, 256]
        pspr = ps("pspr", [128, 32])
        psem = ps("psem", [64, GSL])
        PSTRB = PSTR[:, :].bitcast(BF16)       # [128, 256]

        # semaphores
        dw = sem("dw"); gi = sem("gi")
        pe_s = sem("pe_s"); act_s = sem("act_s"); dve_s = sem("dve_s")
        r_h0 = sem("r_h0"); r_h1 = sem("r_h1"); r_nt = sem("r_nt"); r_em = sem("r_em")
        l_h0 = [sem("l_h0a"), sem("l_h0b")]; l_h1 = [sem("l_h1a"), sem("l_h1b")]
        l_nt = [sem("l_nta"), sem("l_ntb")]; l_em = [sem("l_ema"), sem("l_emb")]
        prep = sem("prep")
        do = [sem("doa"), sem("dob")]

        N_MEMSET = 6   # NT x2, HC0[1], HC1[1], CTC0, CTC1
        N_WLOAD = 22

        # ---- semaphore milestone helpers ----
        def pe_c(ct, k):   # 1=c0stop 2=Tu 3=Tf 4=To 5=c1stop 6-8=T1
            return 10 * ct + k            # 9=embproj 10=E_s

        def pe_d(t, k):    # 1=A3a 2=A3b 3=Tu 4=Tf 5=To 6=C3a 7=C3b
            return 10 * CT + 11 * t + k   # 8=Tu1 9=Tf1 10=To1 11=proj

        def act_c(ct, k):  # 1=tanhg0 2=sigi0 3=sigfo0 4=tanhc0 5-8=L1 9=embtanh
            return 9 * ct + k

        def act_d(t, k):   # 1=tanhg0 2=sigi0 3=sigfo0 4=tanhc0 5-8=L1 9=notetanh
            return 9 * CT + 9 * t + k

        def dve_c(ct, k):  # L0: 1=u 2=fc 3=c 4=h ; L1: 5-8 ; 9=Ecopy
            return 9 * ct + k

        def dve_d(t, k):
            return 9 * CT + 8 * t + k

        def snd_c(ct):     # prior h-staging sends of parity ct%2 (conductor)
            return (ct - ct % 2) // 2

        def snd_d(t):      # conductor + prior decoder sends of parity t%2
            p = t % 2
            return (CT - p + 1) // 2 + (t - p) // 2

        # "previous step" act thresholds with conductor fallback
        def prev_sig0(t):
            return act_d(t - 1, 3) if t >= 1 else act_c(CT - 1, 3)

        def prev_sig1(t):
            return act_d(t - 1, 7) if t >= 1 else act_c(CT - 1, 7)

        def prev_ptanh(t):
            return act_d(t - 1, 9) if t >= 1 else act_c(CT - 1, 9)

        with nc.Block() as block:

            # ================= SYNC: init loads + output stores =============
            @block.sync
            def _(sy):
                loads = [
                    (LAT, latT_d), (H0TA, h0TA_d), (C0T, c0T_d),
                    (WX0, wx0_d), (WH0, wh0_d), (WX1, wx1_d), (WH1, wh1_d),
                    (WDOT, wdoT_d), (WEMB, wemb_d),
                    (WXC0, wxc0_d), (WHC0, whc0_d), (WXC1, wxc1_d),
                    (WHC1, whc1_d), (WCOT, wcoT_d),
                    (B0, b0_d), (B1, b1_d), (BC0, bc0_d), (BC1, bc1_d),
                    (BDO, bdo_d), (BCO, bco_d),
                    (ONES, ones_d), (ID64, id64_d),
                ]
                for dst, src in loads:
                    sy.dma_start(out=dst[:, :], in_=src[:, :]).then_inc(dw, 16)
                for t in range(T):
                    p = t % 2
                    sy.wait_ge(act_s, act_d(t, 9))
                    sy.dma_start(out=out_d[t if full_out else 0],
                                 in_=SNT[p][:, :]).then_inc(do[p], 16)

            # ================= GPSIMD: memsets + exchanges ==================
            @block.gpsimd
            def _(g):
                U32 = mybir.dt.uint32
                for tile in (NT[0], NT[1], HC0[1], HC1[1], CTC0, CTC1):
                    g.memset(tile[:, :].bitcast(U32), 0).then_inc(gi, 1)
                g.wait_ge(gi, N_MEMSET)
                pid = g.partition_id()
                off64 = g.scalar_reg_alu(mybir.AluOpType.mult, pid, SL)
                off32 = g.scalar_reg_alu(mybir.AluOpType.mult, pid, 32)
                np_ = [0]

                def step_bcasts(specs):
                    for stg, gath, goff, gw, rsem, lsem, _, _2 in specs:
                        g.remote_dma_broadcast(
                            out_ap=gath[:, bass.ds(goff, gw)], in_ap=stg[:, :],
                            remote_sem=rsem, local_sem=lsem, rdests=RD,
                        ).then_inc(prep, 1)
                        np_[0] += 1
                    g.wait_ge(prep, np_[0])
                    for _, _1, _2, _3, _4, _5, wait_sem, wait_val in specs:
                        g.wait_ge(wait_sem, wait_val)
                        g.trigger_dma(count=1)

                for ct in range(CT):
                    q = ct % 2
                    step_bcasts([
                        (HSTG0[q], HC0[q], off64, SL, r_h0, l_h0[q], dve_s, dve_c(ct, 4)),
                        (HSTG1[q], HC1[q], off64, SL, r_h1, l_h1[q], dve_s, dve_c(ct, 8)),
                        (SEM_[q], EMBT[q], off32, 32, r_em, l_em[q], act_s, act_c(ct, 9)),
                    ])
                for t in range(T):
                    p = t % 2
                    step_bcasts([
                        (HSTG0[p], HD0[p], off64, SL, r_h0, l_h0[p], dve_s, dve_d(t, 4)),
                        (HSTG1[p], HD1[p], off64, SL, r_h1, l_h1[p], dve_s, dve_d(t, 8)),
                        (SNT[p], NT[p], off32, 32, r_nt, l_nt[p], act_s, act_d(t, 9)),
                    ])

            # ================= TENSOR =======================================
            @block.tensor
            def _(t_):
                pid_t = t_.partition_id()
                par32 = t_.scalar_reg_alu(mybir.AluOpType.bitwise_and, pid_t, 1)
                par32 = t_.scalar_reg_alu(mybir.AluOpType.mult, par32, 32)
                hoff = [t_.scalar_reg_alu(mybir.AluOpType.add, par32, k * SL)
                        for k in range(KT_H)]

                def mm(out, lhsT, rhs, first, last, inc=None, incv=1, **kw):
                    m = t_.matmul(out, lhsT, rhs, start=first, stop=last, **kw)
                    if inc is not None:
                        m.then_inc(inc, incv)
                    return m

                def transposes(S, dve_u, act_sigfo):
                    # T_u (u = i*tanh(g) computed in SBUF by the DVE), then
                    # T_f / T_o from the sigmoid output's f,o columns
                    t_.wait_ge(dve_s, dve_u)
                    t_.transpose(PSTRB[:, 0:64], UR[:, :],
                                 ID64[:, :]).then_inc(pe_s, 1)
                    t_.wait_ge(act_s, act_sigfo)
                    for j, lo in ((1, 256), (2, 384)):
                        t_.transpose(PSTRB[:, 64 * j:64 * (j + 1)],
                                     S[:, lo:lo + 128],
                                     ID64[:, :]).then_inc(pe_s, 1)

                t_.wait_ge(dw, 16 * N_WLOAD)
                t_.wait_ge(gi, N_MEMSET)

                # ---------- conductor ----------
                # X = layer c0 gates; Y = layer c1 gates
                def mm2(psa, psb, lhsT, w, kcol, first, last, inc=None):
                    mm(psa[:, :], lhsT, w[:, kcol:kcol + 256], first, last)
                    mm(psb[:, :], lhsT, w[:, kcol + 256:kcol + 512], first,
                       last, inc=inc)

                def cond_X(ct):
                    q1 = (ct - 1) % 2
                    if ct >= 1:
                        t_.wait_ge(act_s, act_c(ct - 1, 3))
                    mm2(psd0a, psd0b, ONES[:, :], BC0, 0, True, False)
                    for k in range(KT_L):
                        mm2(psd0a, psd0b, LAT[:, SL * k:SL * (k + 1)],
                            WXC0, GSL * k, False, False)
                    if ct >= 1:
                        t_.wait_ge(r_h0, 16 * ct)
                    for k in range(KT_H):
                        mm2(psd0a, psd0b, HC0[q1][:, SL * k:SL * (k + 1)],
                            WHC0, GSL * k, False, k == KT_H - 1,
                            inc=pe_s if k == KT_H - 1 else None)

                def cond_Y12(ct):
                    q1 = (ct - 1) % 2
                    if ct >= 1:
                        t_.wait_ge(act_s, act_c(ct - 1, 7))
                    mm2(psd1a, psd1b, ONES[:, :], BC1, 0, True, False)
                    if ct >= 1:
                        t_.wait_ge(r_h1, 16 * ct)
                    for k in range(KT_H):
                        mm2(psd1a, psd1b, HC1[q1][:, SL * k:SL * (k + 1)],
                            WHC1, GSL * k, False, False)

                def cond_Y3(ct):
                    q = ct % 2
                    t_.wait_ge(r_h0, 16 * (ct + 1))
                    for k in range(KT_H):
                        mm2(psd1a, psd1b, HC0[q][:, SL * k:SL * (k + 1)],
                            WXC1, GSL * k, False, k == KT_H - 1,
                            inc=pe_s if k == KT_H - 1 else None)

                def cond_emb(ct):
                    q = ct % 2
                    if ct >= 1:
                        t_.wait_ge(act_s, act_c(ct - 1, 9))
                    mm(pspr[:, :], BCO[:, :], ONES[:, 0:32], True, False)
                    t_.wait_ge(r_h1, 16 * (ct + 1))
                    for k in range(KT_H):
                        mm(pspr[:, :], WCOT[:, 128 * k:128 * (k + 1)],
                           HC1[q][:, bass.ds(hoff[k], 32)], False,
                           k == KT_H - 1, inc=pe_s if k == KT_H - 1 else None)

                def cond_E(ct):
                    q = ct % 2
                    if ct >= 1:
                        t_.wait_ge(dve_s, dve_c(ct - 1, 9))
                    mm(psem[:, :], ONES[:, :], B0[:, :], True, False)
                    t_.wait_ge(r_em, 16 * (ct + 1))
                    for r in range(4):
                        mm(psem[:, :], EMBT[q][:, SL * r:SL * (r + 1)],
                           WEMB[:, GSL * r:GSL * (r + 1)], False, r == 3,
                           inc=pe_s if r == 3 else None)

                for ct in range(CT):
                    cond_X(ct)
                    transposes(S0, dve_c(ct, 1), act_c(ct, 3))
                    cond_Y12(ct)
                    cond_Y3(ct)
                    transposes(S1, dve_c(ct, 5), act_c(ct, 7))
                    cond_emb(ct)
                    cond_E(ct)

                # ---------- decoder ----------
                def dec_A12(t, lo, hi, est=False):
                    s, n = divmod(t, nnotes)
                    p1 = (t - 1) % 2
                    if est:
                        t_.wait_ge(act_s, prev_sig0(t))
                        if n == 0:
                            t_.wait_ge(dve_s, dve_c(s, 9))
                        else:
                            t_.wait_ge(r_h0, 16 * (CT + t))
                        mm(psd0a[:, :], ID64[:, :],
                           EST[:, GSL * s:GSL * s + 256], True, False)
                        mm(psd0b[:, :], ID64[:, :],
                           EST[:, GSL * s + 256:GSL * (s + 1)], True, False)
                    if n == 0:
                        h0b = (s * 2 + 0) * KT_H * SL
                        stat = lambda k: H0TA[:, h0b + SL * k:h0b + SL * (k + 1)]
                    else:
                        stat = lambda k: HD0[p1][:, SL * k:SL * (k + 1)]
                    for k in range(lo, hi):
                        mm2(psd0a, psd0b, stat(k), WH0, GSL * k, False, False)

                def dec_A3(t, half):   # note-part, gate-col half
                    p1 = (t - 1) % 2
                    if half == 0 and t >= 1:
                        t_.wait_ge(r_nt, 16 * t)
                    psX = psd0a if half == 0 else psd0b
                    c0 = 256 * half
                    for r in range(4):
                        mm(psX[:, :], NT[p1][:, SL * r:SL * (r + 1)],
                           WX0[:, GSL * r + c0:GSL * r + c0 + 256], False,
                           r == 3, inc=pe_s if r == 3 else None)

                def dec_C12(t, lo, hi, bias=False):
                    s, n = divmod(t, nnotes)
                    p1 = (t - 1) % 2
                    if bias:
                        t_.wait_ge(act_s, prev_sig1(t))
                        if n > 0:
                            t_.wait_ge(r_h1, 16 * (CT + t))
                        mm2(psd1a, psd1b, ONES[:, :], B1, 0, True, False)
                    if n == 0:
                        h1b = (s * 2 + 1) * KT_H * SL
                        stat = lambda k: H0TA[:, h1b + SL * k:h1b + SL * (k + 1)]
                    else:
                        stat = lambda k: HD1[p1][:, SL * k:SL * (k + 1)]
                    for k in range(lo, hi):
                        mm2(psd1a, psd1b, stat(k), WH1, GSL * k, False, False)

                def dec_C3(t, half):   # hd0-part, gate-col half
                    p = t % 2
                    if half == 0:
                        t_.wait_ge(r_h0, 16 * (CT + t + 1))
                    psX = psd1a if half == 0 else psd1b
                    c0 = 256 * half
                    for k in range(KT_H):
                        mm(psX[:, :], HD0[p][:, SL * k:SL * (k + 1)],
                           WX1[:, GSL * k + c0:GSL * k + c0 + 256], False,
                           k == KT_H - 1,
                           inc=pe_s if k == KT_H - 1 else None)

                def dec_proj(t):
                    p = t % 2
                    t_.wait_ge(act_s, prev_ptanh(t))
                    mm(pspr[:, :], BDO[:, :], ONES[:, 0:32], True, False)
                    t_.wait_ge(r_h1, 16 * (CT + t + 1))
                    for k in range(KT_H):
                        mm(pspr[:, :], WDOT[:, 128 * k:128 * (k + 1)],
                           HD1[p][:, bass.ds(hoff[k], 32)], False,
                           k == KT_H - 1, inc=pe_s if k == KT_H - 1 else None)

                # software-pipelined issue order: C12(t) fills the L0-elem
                # window, A12(t+1) fills the L1-elem window; gate-col halves
                # let the activations start while the second half's matmuls
                # still stream.
                dec_A12(0, 0, KT_H, est=True)
                for t in range(T):
                    dec_A3(t, 0)                       # pe 1
                    dec_A3(t, 1)                       # pe 2
                    dec_C12(t, 0, 2, bias=True)
                    transposes(S0, dve_d(t, 1), act_d(t, 3))   # pe 3-5
                    dec_C12(t, 2, KT_H)
                    dec_C3(t, 0)                       # pe 6
                    dec_C3(t, 1)                       # pe 7
                    if t + 1 < T:
                        dec_A12(t + 1, 0, 2, est=True)
                    transposes(S1, dve_d(t, 5), act_d(t, 7))   # pe 8-10
                    if t + 1 < T:
                        dec_A12(t + 1, 2, 6)
                    dec_proj(t)                        # pe 11
                    if t + 1 < T:
                        dec_A12(t + 1, 6, KT_H)

            # ================= SCALAR (ACT) =================================
            @block.scalar
            def _(a):
                SIG = mybir.ActivationFunctionType.Sigmoid
                TANH = mybir.ActivationFunctionType.Tanh

                def layer(pe_stop, dve_c_upd, S, CTl, TTl, psa, psb):
                    a.wait_ge(pe_s, pe_stop)
                    a.activation(S[:, 0:128], psa[:, 0:128], TANH).then_inc(act_s, 1)
                    a.activation(S[:, 128:256], psa[:, 128:256], SIG).then_inc(act_s, 1)
                    a.activation(S[:, 256:512], psb[:, :], SIG).then_inc(act_s, 1)
                    a.wait_ge(dve_s, dve_c_upd)
                    a.activation(TTl[:, :], CTl[:, :], TANH).then_inc(act_s, 1)

                for ct in range(CT):
                    q = ct % 2
                    layer(pe_c(ct, 1), dve_c(ct, 3), S0, CTC0, TT0, psd0a, psd0b)
                    layer(pe_c(ct, 5), dve_c(ct, 7), S1, CTC1, TT1, psd1a, psd1b)
                    a.wait_ge(pe_s, pe_c(ct, 9))
                    if snd_c(ct) > 0:
                        a.wait_ge(l_em[q], 16 * snd_c(ct))
                    a.activation(SEM_[q][:, :], pspr[:, :], TANH).then_inc(act_s, 1)
                for t in range(T):
                    p = t % 2
                    for (pa, pb, dv, S, CTl, TTl, psA, psB) in (
                        (pe_d(t, 1), pe_d(t, 2), dve_d(t, 3), S0, CT0, TT0, psd0a, psd0b),
                        (pe_d(t, 6), pe_d(t, 7), dve_d(t, 7), S1, CT1, TT1, psd1a, psd1b),
                    ):
                        a.wait_ge(pe_s, pa)
                        a.activation(S[:, 0:128], psA[:, 0:128], TANH).then_inc(act_s, 1)
                        a.activation(S[:, 128:256], psA[:, 128:256], SIG).then_inc(act_s, 1)
                        a.wait_ge(pe_s, pb)
                        a.activation(S[:, 256:512], psB[:, :], SIG).then_inc(act_s, 1)
                        a.wait_ge(dve_s, dv)
                        a.activation(TTl[:, :], CTl[:, :], TANH).then_inc(act_s, 1)
                    a.wait_ge(pe_s, pe_d(t, 11))
                    if (t - p) // 2 > 0:
                        a.wait_ge(l_nt[p], 16 * ((t - p) // 2))
                        a.wait_ge(do[p], 16 * ((t - p) // 2))
                    a.activation(SNT[p][:, :], pspr[:, :], TANH).then_inc(act_s, 1)

            # ================= VECTOR (DVE) =================================
            @block.vector
            def _(v):
                MUL = mybir.AluOpType.mult
                ADD = mybir.AluOpType.add
                SUB = mybir.AluOpType.subtract
                PU = PSTRB[:, 0:64]
                PF = PSTRB[:, 64:128]
                PO = PSTRB[:, 128:192]

                def layer(sigi_done, tu, dbase, tanh_done, S, CTl, TTl,
                          c_src, c_prev, HSTGt, l_sem, l_val):
                    v.wait_ge(act_s, sigi_done)
                    v.tensor_tensor(UR[:, :], S[:, 128:256], S[:, 0:128],
                                    MUL).then_inc(dve_s, 1)
                    v.wait_ge(pe_s, tu + 1)      # T_f
                    if c_prev > 0:
                        v.wait_ge(dve_s, c_prev)
                    v.tensor_tensor(FCT[:, :], PF, c_src, MUL).then_inc(dve_s, 1)
                    v.wait_ge(dve_s, dbase + 2)
                    v.wait_ge(pe_s, tu)          # T_u
                    v.tensor_tensor(CTl[:, :], PU, FCT[:, :], ADD).then_inc(dve_s, 1)
                    v.wait_ge(act_s, tanh_done)
                    v.wait_ge(pe_s, tu + 2)      # T_o
                    if l_val > 0:
                        v.wait_ge(l_sem, l_val)
                    v.tensor_tensor(HSTGt[:, :], PO, TTl[:, :], MUL).then_inc(dve_s, 1)

                v.wait_ge(gi, N_MEMSET)
                v.wait_ge(dw, 16 * N_WLOAD)
                for ct in range(CT):
                    q = ct % 2
                    layer(act_c(ct, 2), pe_c(ct, 2), dve_c(ct, 0),
                          act_c(ct, 4), S0, CTC0, TT0,
                          CTC0[:, :], dve_c(ct - 1, 3) if ct >= 1 else 0,
                          HSTG0[q], l_h0[q], 16 * snd_c(ct))
                    layer(act_c(ct, 6), pe_c(ct, 6), dve_c(ct, 4),
                          act_c(ct, 8), S1, CTC1, TT1,
                          CTC1[:, :], dve_c(ct - 1, 7) if ct >= 1 else 0,
                          HSTG1[q], l_h1[q], 16 * snd_c(ct))
                    v.wait_ge(pe_s, pe_c(ct, 10))
                    v.tensor_copy(EST[:, GSL * ct:GSL * (ct + 1)], psem[:, :]).then_inc(dve_s, 1)
                for t in range(T):
                    p = t % 2
                    s, n = divmod(t, nnotes)
                    c0src = C0T[:, (s * 2 + 0) * SL:(s * 2 + 1) * SL] if n == 0 else CT0[:, :]
                    c1src = C0T[:, (s * 2 + 1) * SL:(s * 2 + 2) * SL] if n == 0 else CT1[:, :]
                    c0p = 0 if n == 0 else dve_d(t - 1, 3) if t >= 1 else dve_c(CT - 1, 3)
                    c1p = 0 if n == 0 else dve_d(t - 1, 7) if t >= 1 else dve_c(CT - 1, 7)
                    layer(act_d(t, 2), pe_d(t, 3), dve_d(t, 0),
                          act_d(t, 4), S0, CT0, TT0,
                          c0src, c0p, HSTG0[p], l_h0[p], 16 * snd_d(t))
                    layer(act_d(t, 6), pe_d(t, 8), dve_d(t, 4),
                          act_d(t, 8), S1, CT1, TT1,
                          c1src, c1p, HSTG1[p], l_h1[p], 16 * snd_d(t))

    nc.compile()
    return nc


# ======================= host-side preparation =======================

def _gate_slice_ixs(core):
    # column order [g i f o]: g first so ACT can tanh it directly and the
    # DVE chain starts as early as possible
    ix = []
    for gg in (2, 0, 1, 3):
        base = gg * H + core * 128
        ix.extend(range(base, base + 128))
    return np.array(ix)


def _bf16(x):
    import ml_dtypes
    return np.ascontiguousarray(np.asarray(x, np.float32)).astype(ml_dtypes.bfloat16)


def prep_inputs(inputs, nsub=16, nnotes=32):
    f = lambda x: np.asarray(x, dtype=np.float32)
    latent = f(inputs["latent"])
    h0_dec = f(inputs["h0_dec"])[:nsub]
    c0_dec = f(inputs["c0_dec"])[:nsub]

    def pack_k(wT, kt, w=None):
        K, N = wT.shape
        assert K == kt * 128
        out = np.empty((128, kt * N), np.float32)
        for k in range(kt):
            out[:, N * k:N * (k + 1)] = wT[128 * k:128 * (k + 1), :]
        return out

    # h0TA: [(s,l,k) -> 64-col tile], transposed h0_dec
    h0T = np.einsum("slbk->slkb", h0_dec)  # [s, l, 1024, 64]
    h0TA = np.empty((128, nsub * 2 * KT_H * SL), np.float32)
    for s in range(nsub):
        for l in range(2):
            for k in range(KT_H):
                col = ((s * 2 + l) * KT_H + k) * SL
                h0TA[:, col:col + SL] = h0T[s, l, 128 * k:128 * (k + 1), :]

    latT_packed = pack_k(np.ascontiguousarray(latent.T), KT_L)
    ident64 = np.eye(64, dtype=np.float32)
    ones_row = np.ones((1, SL), np.float32)

    Wih_d0, Whh_d0 = f(inputs["Wih_d0"]), f(inputs["Whh_d0"])
    Wih_d1, Whh_d1 = f(inputs["Wih_d1"]), f(inputs["Whh_d1"])
    Wdo, bdo = f(inputs["Wdo"]), f(inputs["bdo"])
    Wih_c0, Whh_c0 = f(inputs["Wih_c0"]), f(inputs["Whh_c0"])
    Wih_c1, Whh_c1 = f(inputs["Wih_c1"]), f(inputs["Whh_c1"])
    Wco, bco = f(inputs["Wco"]), f(inputs["bco"])
    b0_full = f(inputs["bih_d0"]) + f(inputs["bhh_d0"])
    b1_full = f(inputs["bih_d1"]) + f(inputs["bhh_d1"])
    bc0_full = f(inputs["bih_c0"]) + f(inputs["bhh_c0"])
    bc1_full = f(inputs["bih_c1"]) + f(inputs["bhh_c1"])

    Wdo_pad = np.zeros((INPUT_PAD, H), np.float32)
    Wdo_pad[:INPUT] = Wdo
    bdo_pad = np.zeros(INPUT_PAD, np.float32)
    bdo_pad[:INPUT] = bdo

    in_maps = []
    for core in range(NC):
        ix = _gate_slice_ixs(core)

        def slc(w):
            return np.ascontiguousarray(w[ix, :].T.astype(np.float32))

        wx0_full = np.zeros((INPUT_PAD, GSL), np.float32)
        wx0_full[:INPUT] = slc(Wih_d0[:, :INPUT])
        wemb_full = slc(Wih_d0[:, INPUT:INPUT + 512])

        pair = core // 2   # note/emb row block [128*pair, 128*(pair+1))
        m = {
            "latT": latT_packed,
            "h0TA": h0TA,
            "c0T": np.concatenate(
                [c0_dec[s, l, :, core * 128:(core + 1) * 128].T
                 for s in range(nsub) for l in range(2)], axis=1),
            "wx0": pack_k(wx0_full, 4),
            "wh0": pack_k(slc(Whh_d0), KT_H),
            "wx1": pack_k(slc(Wih_d1), KT_H),
            "wh1": pack_k(slc(Whh_d1), KT_H),
            "wdoT": pack_k(np.ascontiguousarray(
                Wdo_pad[128 * pair:128 * (pair + 1), :].T), KT_H),
            "wemb": pack_k(wemb_full, 4),
            "wxc0": pack_k(slc(Wih_c0), KT_L),
            "whc0": pack_k(slc(Whh_c0), KT_H),
            "wxc1": pack_k(slc(Wih_c1), KT_H),
            "whc1": pack_k(slc(Whh_c1), KT_H),
            "wcoT": pack_k(np.ascontiguousarray(
                Wco[128 * pair:128 * (pair + 1), :].T), KT_H),
            "b0r": b0_full[ix][None, :],
            "b1r": b1_full[ix][None, :],
            "bc0r": bc0_full[ix][None, :],
            "bc1r": bc1_full[ix][None, :],
            "bdoc": bdo_pad[128 * pair:128 * (pair + 1)][None, :],
            "bcoc": bco[128 * pair:128 * (pair + 1)][None, :],
            "onesr": ones_row,
            "id64": ident64,
        }
        mm = {k: _bf16(v) for k, v in m.items() if k != "c0T"}
        mm["c0T"] = np.ascontiguousarray(m["c0T"], dtype=np.float32)
        in_maps.append(mm)
    return in_maps


def assemble_output(results, nsub=16, nnotes=32):
    T = nsub * nnotes
    full = np.zeros((T, INPUT_PAD, B), np.float32)
    for c in range(NC):
        o = np.asarray(results[c]["out"], dtype=np.float32)  # [T, 128, 32]
        pair, half = c // 2, c % 2
        full[:, 128 * pair:128 * (pair + 1), 32 * half:32 * (half + 1)] = o
    return np.ascontiguousarray(full[:, :INPUT, :].transpose(2, 0, 1))


_CACHED = {}


def kernel(**inputs) -> np.ndarray:
    from concourse.bass_utils import run_bass_kernel_spmd
    nsub, nnotes = 16, 32
    key = (nsub, nnotes)
    if key not in _CACHED:
        _CACHED[key] = build(nsub, nnotes)
    nc = _CACHED[key]
    in_maps = prep_inputs(inputs, nsub, nnotes)
    res = run_bass_kernel_spmd(nc, in_maps, core_ids=list(range(NC)))
    return assemble_output(res.results, nsub, nnotes)
